# revision 15
# baseline (speedup 1.0000x reference)
"""Trainium2 Bass kernel for nn_AbsoluteHeadProbEncoder.

Math (mask all-ones, STEP=1, DAMP=0, REG=1), per batch z:
  qz = x
  repeat 4x:
    S = softmax(qz, axis=-1)                          # [L, d]
    per head c:
      W1T[b,i] = sum_a (T[a,b,c]*d) S[i,a]            # f32r matmul
      M[i,j] = sum_b W1T[b,i] ST[b,j]  - 1e9*I        # f32r + bf16 negeye
      E = exp(M - rowmax)  (f16)                      # unnormalized
      ET = DMA-xbar transpose of E                    # off-PE
      P[j,(c,a)] = S @ Tc^T; R[j,(c,a)] = S @ Tc      # bf16 single
      T1[i,a|rowsum] = sum_jb ET_jb^T @ [P|1]         # T1-direct, [i,a] layout
      acc += T1 * (1/rowsum)
      rw = R * (1/rowsum);  T2acc[a,i] += rw^T @ E    # accum over heads
    qz = acc + T2acc^T   (acc starts at xc)

Sharding: batch z on core pair (z, z+4); heads 0-3 on core z, 4-7 on
core z+4 (ternary sliced per core). Per iteration the partial message
(xc + own-head T1/T2; xc = x on cores 0-3 else 0) is AllGathered over
the pair and summed, so both cores hold the full qz.

Precision: f32r (~1.6e-4) for W1/M, bf16 single for P/R, f16 E/pw/rw.
"""
import os
import sys
import numpy as np

if '/opt/trn_rl_repo' not in sys.path:
    sys.path.insert(0, '/opt/trn_rl_repo')

import concourse.bass as bass
import concourse.tile as tile
from concourse import mybir
from concourse.bass_utils import run_bass_kernel_spmd

B, L, D, H, NITER = 4, 512, 64, 8, 4
NB = L // 128            # 4 i/j blocks
NEG = 1e9
SPLIT = os.environ.get("KSPLIT", "1") != "0"
H_LOC = H // 2 if SPLIT else H
GROUPS = [[0, 4], [1, 5], [2, 6], [3, 7]]

# blob layout (fp32):
#   x[0:256] | xc[256:512] | ident[512:640] | TC | TP | TR  (H_LOC heads each)
XC0, XCC0, IC0 = 0, 256, 512
TW = 64 * (H // 2 if SPLIT else H)
TC0, TP0, TR0 = 640, 640 + TW, 640 + 2 * TW
BLOBW = 640 + 3 * TW

_SKIP_FIX = None


def _fix_waits(nc, max_inline=1):
    """Hoist excess per-instruction sem waits into standalone event-sem
    instructions (walrus encodes limited sync-wait slots per instruction)."""
    global _SKIP_FIX
    if _SKIP_FIX is None:
        _SKIP_FIX = (
            mybir.InstEventSemaphore, mybir.InstAllEngineBarrier,
            mybir.InstUnconditionalBranch, mybir.InstCompareAndBranch,
            mybir.InstIndirectBranch, mybir.InstBranchHint, mybir.InstHalt,
        )
    n = 0
    cnt = [0]
    for f in nc.m.functions:
        for bb in f.blocks:
            out = []
            for ins in bb.instructions:
                si = ins.sync_info
                if (si is not None and si.on_wait and len(si.on_wait) > max_inline
                        and not isinstance(ins, _SKIP_FIX)):
                    waits = list(si.on_wait)
                    extra, keep = waits[:-max_inline], waits[-max_inline:]
                    for w in extra:
                        cnt[0] += 1
                        ev = mybir.InstEventSemaphore(
                            name=f"I-waitfix-{cnt[0]}", ins=[], outs=[],
                            sync_info=mybir.SyncInfo(on_wait=[w], on_update=[]))
                        ev.engine = ins.engine
                        out.append(ev)
                    ins.sync_info = mybir.SyncInfo(
                        on_wait=keep, on_update=list(si.on_update or []))
                    n += 1
                out.append(ins)
            bb.instructions = out
    return n


def build_nc():
    f32 = mybir.dt.float32
    f32r = mybir.dt.float32r
    bf16 = mybir.dt.bfloat16
    f16 = mybir.dt.float16
    AF = mybir.ActivationFunctionType
    AX = mybir.AxisListType
    OP = mybir.AluOpType
    HL = H_LOC

    nc = bass.Bass(num_devices=8 if SPLIT else None)
    blob_ext = nc.declare_dram_parameter("blob", [128, BLOBW], f32, isOutput=False)
    out_ext = nc.declare_dram_parameter("out", [128, NB, D], f32, isOutput=True)

    with tile.TileContext(nc) as tc:
        with tc.tile_pool(name="const", bufs=1) as const, \
             tc.tile_pool(name="qzp", bufs=2) as qzp, \
             tc.tile_pool(name="smp", bufs=2) as smp, \
             tc.tile_pool(name="stp", bufs=2) as stp, \
             tc.tile_pool(name="w1p", bufs=2) as w1p, \
             tc.tile_pool(name="pwp", bufs=2) as pwp, \
             tc.tile_pool(name="ep", bufs=4) as ep, \
             tc.tile_pool(name="etp", bufs=6) as etp, \
             tc.tile_pool(name="rwp", bufs=4) as rwp, \
             tc.tile_pool(name="tiny", bufs=8) as tiny, \
             tc.tile_pool(name="dram", bufs=2 * NITER, space="DRAM") as dram, \
             tc.tile_pool(name="ps_m", bufs=2, space="PSUM") as ps_m, \
             tc.tile_pool(name="ps_t1", bufs=1, space="PSUM") as ps_t1, \
             tc.tile_pool(name="ps_t2", bufs=2, space="PSUM") as ps_t2, \
             tc.tile_pool(name="ps_s", bufs=2, space="PSUM") as ps_s, \
             tc.tile_pool(name="ps_x", bufs=1, space="PSUM") as ps_x:

            blob = const.tile([128, BLOBW], f32)
            nc.sync.dma_start(out=blob, in_=blob_ext[:, :])
            x_sb = blob[:, XC0:XC0 + NB * D].rearrange("p (nb d) -> p nb d", nb=NB)
            xc_sb = blob[:, XCC0:XCC0 + NB * D].rearrange("p (nb d) -> p nb d",
                                                          nb=NB)
            ident = blob[:, IC0:IC0 + 128]

            # one-time consts
            negeye = const.tile([128, 128], bf16)
            nc.vector.tensor_scalar_mul(negeye, ident, -NEG)
            eye_bf = const.tile([128, 128], bf16)
            nc.vector.tensor_copy(eye_bf, ident)
            tc_r = const.tile([64, HL * D], f32r)
            nc.scalar.copy(tc_r, blob[0:64, TC0:TC0 + HL * D])
            tp_r = const.tile([64, HL * D], f32r)
            nc.scalar.copy(tp_r, blob[0:64, TP0:TP0 + HL * D])
            tr_r = const.tile([64, HL * D], f32r)
            nc.scalar.copy(tr_r, blob[0:64, TR0:TR0 + HL * D])

            qz_prev = None
            for it in range(NITER):
                # ---- softmax(qz) over d -> s_sb [128, nb, 64] fp32
                src = x_sb if it == 0 else qz_prev
                negq = tiny.tile([128, NB], f32, tag="negq")
                nc.vector.tensor_reduce(negq, src, axis=AX.X, op=OP.max, negate=True)
                expq = smp.tile([128, NB, D], f32, tag="expq")
                rsq = tiny.tile([128, NB], f32, tag="rsq")
                for ib in range(NB):
                    nc.scalar.activation(expq[:, ib, :], src[:, ib, :], AF.Exp,
                                         bias=negq[:, ib:ib + 1], scale=1.0,
                                         accum_out=rsq[:, ib:ib + 1])
                rcq = tiny.tile([128, NB], f32, tag="rcq")
                nc.vector.reciprocal(rcq, rsq)
                s_sb = smp.tile([128, NB, D], f32, tag="s_sb")
                for ib in range(NB):
                    nc.vector.tensor_scalar_mul(s_sb[:, ib, :], expq[:, ib, :],
                                                rcq[:, ib:ib + 1])

                # ---- ST = S^T [64, 512]: f32r for W1/M, bf16 for P/R
                stps = ps_s.tile([64, 512], f32, tag="ps_small")
                for ib in range(NB):
                    nc.tensor.transpose(stps[:, 128 * ib:128 * (ib + 1)],
                                        s_sb[:, ib, :], ident)
                st = stp.tile([64, 512], f32r, tag="st")
                nc.scalar.copy(st, stps)


                # ---- W1T per head: w1sb[c] = [b=64, i=512] f32r
                w1sb = w1p.tile([64, HL, 512], f32r, tag="w1sb")
                for c in range(HL):
                    w1ps = ps_s.tile([64, 512], f32, tag="ps_small")
                    nc.tensor.matmul(w1ps, tc_r[:, D * c:D * (c + 1)], st,
                                     start=True, stop=True, skip_group_check=True)
                    if c % 2 == 0:
                        nc.scalar.copy(w1sb[:, c, :], w1ps)
                    else:
                        nc.vector.tensor_copy(w1sb[:, c, :], w1ps)

                # ---- P/R all local heads (bf16 single): pw f16, r_sb fp32
                pw = pwp.tile([128, NB, HL, D + 1], f16, tag="pw")
                nc.vector.memset(pw[:, :, :, D:D + 1], 1.0)
                r_sb = pwp.tile([128, NB, HL, D], f32, tag="r_sb")
                for jb in range(NB):
                    pps = ps_s.tile([128, HL * D], f32, tag="ps_small")
                    nc.tensor.matmul(pps, st[:, 128 * jb:128 * (jb + 1)], tp_r,
                                     start=True, stop=True, skip_group_check=True)
                    nc.scalar.copy(pw[:, jb, :, 0:D],
                                   pps.rearrange("p (c a) -> p c a", c=HL))
                    rps = ps_s.tile([128, HL * D], f32, tag="ps_small")
                    nc.tensor.matmul(rps, st[:, 128 * jb:128 * (jb + 1)], tr_r,
                                     start=True, stop=True, skip_group_check=True)
                    nc.scalar.copy(r_sb[:, jb, :, :],
                                   rps.rearrange("p (c a) -> p c a", c=HL))

                # ---- per-head pipeline
                acc = qzp.tile([128, NB, D], f32, tag="acc")
                t2a0 = ps_t2.tile([64, 512], f32, tag="t2acc")
                t2a1 = ps_t2.tile([64, 512], f32, tag="t2acc")
                t2a = [t2a0, t2a1]
                half = max(HL // 2, 1)
                for c in range(HL):
                    negm = tiny.tile([128, NB], f32, tag="negm")
                    e_raw = ep.tile([128, NB, 512], f16, tag="e_raw")
                    for ib in range(NB):
                        mps = ps_m.tile([128, 512], f32, tag="mps")
                        nc.tensor.matmul(mps,
                                         w1sb[:, c, 128 * ib:128 * (ib + 1)],
                                         st, start=True, stop=False,
                                         skip_group_check=True)
                        nc.tensor.matmul(mps[:, 128 * ib:128 * (ib + 1)],
                                         negeye, eye_bf, start=False, stop=True,
                                         skip_group_check=True)
                        nc.vector.tensor_reduce(negm[:, ib:ib + 1], mps,
                                                axis=AX.X, op=OP.max, negate=True)
                        nc.scalar.activation(e_raw[:, ib, :], mps, AF.Exp,
                                             bias=negm[:, ib:ib + 1], scale=1.0)
                    # ET via DMA xbar transpose: et_ib[p, m, l] = E[128ib+l, 128m+p]
                    et_l = []
                    for ib in range(NB):
                        et = etp.tile([128, NB, 128], f16, tag=f"et{ib}")
                        nc.sync.dma_start(out=et, in_=e_raw[:, ib, :],
                                          transpose=True)
                        et_l.append(et)
                    # T1-direct: t1ps[:, ib, :] = sum_jb ET[jb,ib]^T @ pw[jb]
                    t1ps = ps_t1.tile([128, NB, D + 1], f32, tag="t1ps")
                    for ib in range(NB):
                        for jb in range(NB):
                            nc.tensor.matmul(t1ps[:, ib, :],
                                             et_l[ib][:, jb, :],
                                             pw[:, jb, c, :],
                                             start=(jb == 0), stop=(jb == NB - 1),
                                             skip_group_check=True)
                    # normalize + accumulate; rw = R * rc2
                    rc2 = tiny.tile([128, NB], f32, tag="rc2")
                    nc.vector.reciprocal(rc2, t1ps[:, :, D:D + 1].rearrange(
                        "p nb one -> p (nb one)"))
                    base = xc_sb if c == 0 else acc
                    rw = rwp.tile([128, NB, D], f16, tag="rw")
                    for ib in range(NB):
                        nc.vector.tensor_scalar_mul(rw[:, ib, :],
                                                    r_sb[:, ib, c, :],
                                                    rc2[:, ib:ib + 1])
                    for ib in range(NB):
                        nc.vector.scalar_tensor_tensor(
                            acc[:, ib, :], t1ps[:, ib, 0:D], rc2[:, ib:ib + 1],
                            base[:, ib, :], op0=OP.mult, op1=OP.add)
                    # T2: accumulate over (c, jb), split across two psum banks
                    t2acc = t2a[c // half]
                    first, last = (c % half == 0), (c % half == half - 1)
                    for jb in range(NB):
                        nc.tensor.matmul(t2acc, rw[:, jb, :], e_raw[:, jb, :],
                                         start=(first and jb == 0),
                                         stop=(last and jb == NB - 1),
                                         skip_group_check=True)
                if HL == 1:
                    nc.vector.memset(t2a1, 0.0)

                # ---- combine partial: part = acc + (t2a0 + t2a1)^T
                t2sb = smp.tile([64, 512], f32, tag="t2sb")
                nc.scalar.copy(t2sb, t2a[0])
                nc.vector.tensor_add(t2sb, t2sb, t2a[1])
                t2t = ps_x.tile([128, NB, D], f32, tag="ps_t2t")
                for ib in range(NB):
                    nc.tensor.transpose(t2t[:, ib, :],
                                        t2sb[:, 128 * ib:128 * (ib + 1)],
                                        ident[0:64, 0:64])
                qz_new = qzp.tile([128, NB, D], f32, tag="qz")
                nc.vector.tensor_add(qz_new, acc, t2t)

                if SPLIT and it < NITER - 1:
                    # exchange partials over the pair and sum
                    bin_ = dram.tile([128, NB * D], f32)
                    bout = dram.tile([2, 128, NB * D], f32)
                    nc.gpsimd.dma_start(out=bin_,
                                        in_=qz_new.rearrange("p nb d -> p (nb d)"))
                    nc.gpsimd.collective_compute(
                        "AllGather", OP.bypass, replica_groups=GROUPS,
                        ins=[bin_[:, :].opt()], outs=[bout[:, :, :].opt()])
                    both = qzp.tile([128, 2, NB * D], f32, tag="both")
                    nc.sync.dma_start(out=both, in_=bout.rearrange(
                        "t p w -> p t w"))
                    qz_full = qzp.tile([128, NB, D], f32, tag="qzf")
                    nc.vector.tensor_add(
                        qz_full,
                        both[:, 0, :].rearrange("p (nb d) -> p nb d", nb=NB),
                        both[:, 1, :].rearrange("p (nb d) -> p nb d", nb=NB))
                    qz_prev = qz_full
                else:
                    qz_prev = qz_new

            nc.sync.dma_start(out=out_ext[:, :, :], in_=qz_prev)

    _fix_waits(nc)
    return nc


_NC_CACHE = None
_LAST_RESULTS = None


def _np_reference(x, mask, ternary):
    """Numpy fallback (general mask), used only if mask isn't all-ones."""
    O = dict(optimize=True)
    valid = (mask != 0)
    v1 = valid[:, :, None]
    pinv = ~(valid[:, None, :, None] & valid[:, None, None, :])
    diag = np.eye(L, dtype=np.float32) * NEG

    def sm(a):
        m = a.max(-1, keepdims=True)
        e = np.exp(a - m)
        return e / e.sum(-1, keepdims=True)

    qz = np.where(v1, x, 0.0).astype(np.float32)
    for it in range(NITER):
        nz = sm(qz)
        qz = nz
        qz = np.where(v1, qz, 0.0)
        msg_F = np.einsum('zia,zjb,abc->zcij', qz, qz, ternary, **O)
        qh = msg_F * D - diag
        qh = np.where(np.broadcast_to(pinv.transpose(0, 3, 1, 2), qh.shape), -NEG, qh)
        qh = sm(qh)
        G = (np.einsum('zjb,zcij,abc->zia', qz, qh, ternary, **O)
             + np.einsum('zjb,zcji,bac->zia', qz, qh, ternary, **O))
        qz = (x + G).astype(np.float32)
    return qz


def kernel(x, mask, ternary):
    x = np.ascontiguousarray(x, dtype=np.float32)
    ternary = np.ascontiguousarray(ternary, dtype=np.float32)
    if not np.all(np.asarray(mask) != 0):
        return _np_reference(x, np.asarray(mask), ternary)

    global _NC_CACHE
    if _NC_CACHE is None:
        _NC_CACHE = build_nc()
    nc = _NC_CACHE

    ident = np.eye(128, dtype=np.float32)
    tc_host = np.transpose(ternary, (0, 2, 1)).reshape(64, H * D)  # [a,(c,b)]
    tp = np.transpose(ternary, (1, 2, 0)).reshape(64, H * D)       # [b,(c,a)]=T[a,b,c]
    tr = np.transpose(ternary, (0, 2, 1)).reshape(64, H * D)       # [b,(c,a)]=T[b,a,c]

    in_maps = []
    for core in range(8):
        z = core % B
        half = core // B if SPLIT else 0
        lo, hi = half * H_LOC * D, (half + 1) * H_LOC * D
        blob = np.zeros((128, BLOBW), np.float32)
        xr = x[z].reshape(NB, 128, D).transpose(1, 0, 2).reshape(128, NB * D)
        blob[:, XC0:XC0 + NB * D] = xr
        if (not SPLIT) or half == 0:
            blob[:, XCC0:XCC0 + NB * D] = xr
        blob[:, IC0:IC0 + 128] = ident
        blob[0:64, TC0:TC0 + H_LOC * D] = tc_host[:, lo:hi] * float(D)
        blob[0:64, TP0:TP0 + H_LOC * D] = tp[:, lo:hi]
        blob[0:64, TR0:TR0 + H_LOC * D] = tr[:, lo:hi]
        in_maps.append({"blob": blob})

    global _LAST_RESULTS
    res = run_bass_kernel_spmd(nc, in_maps, core_ids=list(range(8)))
    _LAST_RESULTS = res
    out = np.empty((B, L, D), np.float32)
    for z in range(B):
        o = res.results[z]["out"]            # [128, NB, D] (partial if SPLIT)
        if SPLIT:
            o = o + res.results[z + 4]["out"]
        out[z] = o.transpose(1, 0, 2).reshape(L, D)
    return out
